# revision 26
# baseline (speedup 1.0000x reference)
"""Bass/Trainium2 kernel for nn_ExaoneMoEAttention (sliding-window GQA attention).

Strategy (8 NeuronCores, tensor-parallel over heads):
  - core c owns q heads 4c..4c+3 and kv head c (w_qkv column shard [4096, 768]),
    plus w_o rows 512c..512c+512 ([512, 4096]).
  - hidden replicated, host-transposed/blocked; fp32r matmuls throughout.
  - Phase A (QKV proj): per 128-row t-tile, hidT tiles are the stationary
    operand and w_qkv columns the moving operand (qkv lands in [t, c] psum);
    RMSNorm stats via ACT Square+accum_out on the free dim; the normalized
    q/k head tiles are PE-transposed to [d, t] strips resident in SBUF, with
    norm-weight (and softmax scale for q) folded into the transpose
    evacuation; RoPE via host cos/sin tables; v needs no transpose.
  - Phase B: scoresT[s, t] tiles of [128, 512]; sliding window (1024) +
    causal handled block-sparsely (<=12 key tiles per 512-wide q chunk) with
    multiplicative 0/1 masks; softmax without max-subtraction (RMSNorm bounds
    |score| <= sqrt(D)); exp-sum via ones-matmul; two GQA heads pipelined
    together; normalization by broadcast reciprocal, pipelined into the next
    head-pair.
  - o_proj per 512-row slab, then bf16 ReduceScatter(add) over the 8 cores
    per slab (overlaps later slabs); host concatenates the 8 row-shards.
"""

import ml_dtypes
import numpy as np

import concourse.bass as bass
import concourse.mybir as mybir
import concourse.tile as tile
from concourse import bacc
from concourse.bass_utils import run_bass_kernel_spmd
from concourse.masks import make_identity

F32 = mybir.dt.float32
F32R = mybir.dt.float32r
BF16 = mybir.dt.bfloat16
AF = mybir.ActivationFunctionType

N_CORES = 8
T = 2048
HID = 4096
H = 32
HKV = 8
D = 128
EPS = 1e-5
THETA = 1e6
WINDOW = 1024

HL = H // N_CORES          # 4 local q heads
NT = T // 128              # 16 t/s tiles
KO = HID // 128            # 32 k-subtiles in projection
QC = 512                   # q chunk in attention phase
N_QC = T // QC             # 4
ECH = 512                  # o_proj e-chunk
N_ECH = HID // ECH         # 8

MASK_DELTAS = [0, -128, -256, -384, 640, 768, 896, 1024]
MASK_IDX = {d: i for i, d in enumerate(MASK_DELTAS)}


def _build():
    nc = bacc.Bacc(num_devices=N_CORES)

    # hidT4[ki, tt, ko, j] = hidden[tt*128+j, ko*128+ki]
    hidT = nc.declare_dram_parameter("hidT", [128, NT, KO, 128], BF16, isOutput=False)
    wq = nc.declare_dram_parameter("wq", [128, KO, (HL + 2) * D], BF16, isOutput=False)
    wo = nc.declare_dram_parameter("wo", [128, HL, HID], BF16, isOutput=False)
    cs2 = nc.declare_dram_parameter("cs2", [128, T], F32, isOutput=False)
    sn2s = nc.declare_dram_parameter("sn2s", [128, T], F32, isOutput=False)
    masks = nc.declare_dram_parameter("masks", [128, len(MASK_DELTAS), QC], BF16, isOutput=False)
    qwv = nc.declare_dram_parameter("qwv", [D, 1], F32, isOutput=False)
    kwv = nc.declare_dram_parameter("kwv", [D, 1], F32, isOutput=False)
    onc_d = nc.declare_dram_parameter("onc", [128, 1], BF16, isOutput=False)
    onr_d = nc.declare_dram_parameter("onr", [1, 128], F32R, isOutput=False)
    out_p = nc.declare_dram_parameter("out", [N_QC, QC // N_CORES, HID], BF16, isOutput=True)

    with tile.TileContext(nc) as tc:
        with tc.tile_pool(name="persistA", bufs=1) as pA:
            kT = pA.tile([128, T], F32R)                     # rope'd k, [d, s]
            qT = [pA.tile([128, T], F32R, name=f"qT{h}") for h in range(HL)]
            vnat = pA.tile([128, NT, D], BF16)               # v in [s, d] tiles
            onc = pA.tile([128, 1], BF16)
            onr = pA.tile([1, 128], F32R)
            ident = pA.tile([128, 128], BF16)
            make_identity(nc, ident[:])
            nc.sync.dma_start(out=onc[:], in_=onc_d[:])
            nc.sync.dma_start(out=onr[:], in_=onr_d[:])

            # ---------------- Phase A: QKV projection + norm + rope ----------
            with (
                tc.tile_pool(name="wpool", bufs=1) as wpool,
                tc.tile_pool(name="hidp", bufs=3) as hidp,
                tc.tile_pool(name="cspool", bufs=2) as cspool,
                tc.tile_pool(name="tmpA", bufs=6) as tmpA,
                tc.tile_pool(name="stA", bufs=6) as stA,
                tc.tile_pool(name="miscA", bufs=1) as miscA,
                tc.tile_pool(name="psq", bufs=3, space="PSUM") as psq_p,
                tc.tile_pool(name="psvt", bufs=2, space="PSUM") as psvt_p,
            ):
                w_grp = []
                for g in range(4):
                    wt = wpool.tile([128, KO // 4, (HL + 2) * D], BF16, name=f"w{g}")
                    nc.sync.dma_start(out=wt[:], in_=wq[:, g * (KO // 4):(g + 1) * (KO // 4), :])
                    w_grp.append(wt)
                qw_sb = miscA.tile([D, 1], F32)
                kw_sb = miscA.tile([D, 1], F32)
                eps_sb = miscA.tile([128, 1], F32)
                nc.sync.dma_start(out=qw_sb[:], in_=qwv[:])
                nc.sync.dma_start(out=kw_sb[:], in_=kwv[:])
                nc.vector.memset(eps_sb[:], EPS)

                pending_post = []

                def flush_post(keep=0):
                    while len(pending_post) > keep:
                        pending_post.pop(0)()

                for tt in range(NT):
                    tsl = slice(tt * 128, (tt + 1) * 128)
                    hid_t = hidp.tile([128, KO, 128], BF16, tag="hid")
                    nc.sync.dma_start(out=hid_t[:], in_=hidT[:, tt])
                    cs_t = cspool.tile([128, 128], F32, tag="cs")
                    sn_t = cspool.tile([128, 128], F32, tag="sn")
                    nc.sync.dma_start(out=cs_t[:], in_=cs2[:, tsl])
                    nc.sync.dma_start(out=sn_t[:], in_=sn2s[:, tsl])

                    # qkv[t, c] for this t-tile: [128, 512] + [128, 256] psums
                    pq = psq_p.tile([128, 4 * D], F32, tag="pq")
                    pq2 = psq_p.tile([128, 2 * D], F32, tag="pq2")
                    for ko in range(KO):
                        lhsT = hid_t[:, ko, :]
                        wg = w_grp[ko // (KO // 4)]
                        nc.tensor.matmul(
                            pq[:], lhsT, wg[:, ko % (KO // 4), 0:4 * D],
                            start=(ko == 0), stop=(ko == KO - 1),
                        )
                        nc.tensor.matmul(
                            pq2[:], lhsT, wg[:, ko % (KO // 4), 4 * D:6 * D],
                            start=(ko == 0), stop=(ko == KO - 1),
                        )
                    flush_post(keep=1)

                    def make_post(tt=tt, pq=pq, pq2=pq2, tsl=tsl, cs_t=cs_t, sn_t=sn_t):
                        def _post():
                            # stage-parallel across the 5 normed heads so the
                            # ACT/DVE chains pipeline instead of serializing
                            srcs = [pq[:, m * D:(m + 1) * D] for m in range(HL)] + [pq2[:, 0:D]]
                            var, sd, rstd, ev, tp, qd, qsw = [], [], [], [], [], [], []
                            for m in range(HL + 1):
                                sqd = tmpA.tile([128, D], F32, tag="sqd", name="sqd")
                                var.append(stA.tile([128, 1], F32, tag="var", name="var"))
                                nc.scalar.activation(sqd[:], srcs[m], AF.Square, accum_out=var[m][:])
                            for m in range(HL + 1):
                                sd.append(stA.tile([128, 1], F32, tag="sd", name="sd"))
                                nc.scalar.activation(sd[m][:], var[m][:], AF.Sqrt, scale=1.0 / D, bias=eps_sb[:])
                            for m in range(HL + 1):
                                rstd.append(stA.tile([128, 1], F32, tag="rstd", name="rstd"))
                                nc.vector.reciprocal(rstd[m][:], sd[m][:])
                            for m in range(HL + 1):
                                ev.append(tmpA.tile([128, D], BF16, tag="ev", name="ev"))
                                nc.scalar.activation(ev[m][:], srcs[m], AF.Copy, scale=rstd[m][:])
                            for m in range(HL + 1):
                                tp.append(psvt_p.tile([128, 128], BF16, tag="tp", name="tp"))
                                nc.tensor.transpose(tp[m][:], ev[m][:], ident[:])
                            for m in range(HL + 1):
                                qd.append(tmpA.tile([128, D], F32, tag="qd", name="qd"))
                                nc.scalar.activation(
                                    qd[m][:], tp[m][:], AF.Copy,
                                    scale=(qw_sb[:] if m < HL else kw_sb[:]),
                                )
                            for m in range(HL + 1):
                                qsw.append(tmpA.tile([128, D], F32, tag="qsw", name="qsw"))
                                nc.vector.tensor_copy(qsw[m][0:64, :], qd[m][64:128, :])
                                nc.vector.tensor_copy(qsw[m][64:128, :], qd[m][0:64, :])
                            for m in range(HL + 1):
                                nc.vector.tensor_mul(qd[m][:], qd[m][:], cs_t[:])
                                nc.vector.tensor_mul(qsw[m][:], qsw[m][:], sn_t[:])
                                dst = qT[m][:, tsl] if m < HL else kT[:, tsl]
                                nc.vector.tensor_add(dst, qd[m][:], qsw[m][:])
                            nc.vector.tensor_copy(vnat[:, tt, :], pq2[:, D:2 * D])
                        return _post

                    pending_post.append(make_post())
                flush_post()

            # ---------------- Phase B: attention + o_proj + reduce-scatter ---
            with (
                tc.tile_pool(name="persistB", bufs=1) as pB,
                tc.tile_pool(name="exp", bufs=8) as exp_p,
                tc.tile_pool(name="stB", bufs=2) as stB,
                tc.tile_pool(name="ostg", bufs=8) as ostg_p,
                tc.tile_pool(name="cvp", bufs=1) as cvp,
                tc.tile_pool(name="psA", bufs=4, space="PSUM") as psA_p,
                tc.tile_pool(name="psav", bufs=2, space="PSUM") as psav_p,
                tc.tile_pool(name="pssum", bufs=2, space="PSUM") as pssum_p,
                tc.tile_pool(name="dramB", bufs=1, space="DRAM") as dramB,
            ):
                attnT = pB.tile([128, HL, T], BF16)
                wo_sb = pB.tile([128, HL, HID], BF16)
                mask_sb = pB.tile([128, len(MASK_DELTAS), QC], BF16)
                nc.sync.dma_start(out=mask_sb[:], in_=masks[:])
                nc.gpsimd.dma_start(out=wo_sb[:], in_=wo[:])

                partial = [
                    dramB.tile([QC, HID], BF16, name=f"partial{qc}") for qc in range(N_QC)
                ]
                rs_out = [
                    dramB.tile([QC // N_CORES, HID], BF16, name=f"rsout{qc}")
                    for qc in range(N_QC)
                ]
                rs_half = [
                    dramB.tile([QC // (2 * N_CORES), HID], BF16, name=f"rshalf{i}")
                    for i in range(4)
                ]

                pending_norm = [None]

                def flush_norm():
                    if pending_norm[0] is not None:
                        pending_norm[0]()
                        pending_norm[0] = None

                flush_norm2 = flush_norm

                # o_proj emitted as small PE groups, interleaved into the next
                # slab's attention loop as filler work so the PE stream stays
                # dense (keeps the HAM clock-gate warm)
                oproj_q = []

                def emit_oproj_group():
                    if oproj_q:
                        oproj_q.pop(0)()

                def queue_oproj(qc):
                    def emit_rs(qc, half):
                        # half: None = whole slab; 0/1 = 256-row half of the
                        # final slab (lets RS(3a) overlap o_proj(3b), halving
                        # the un-overlapped tail collective)
                        if half is None:
                            nc.gpsimd.collective_compute(
                                "ReduceScatter",
                                mybir.AluOpType.add,
                                replica_groups=[list(range(N_CORES))],
                                ins=[partial[qc][:]],
                                outs=[rs_out[qc][:]],
                            )
                            # same gpsimd queue as the RS, so this wait can't
                            # poison other engines
                            nc.gpsimd.dma_start(out=out_p[qc], in_=rs_out[qc][:])
                        else:
                            hrows = QC // 2
                            rh = rs_half[(qc % 2) * 2 + half]
                            nc.gpsimd.collective_compute(
                                "ReduceScatter",
                                mybir.AluOpType.add,
                                replica_groups=[list(range(N_CORES))],
                                ins=[partial[qc][half * hrows:(half + 1) * hrows, :]],
                                outs=[rh[:]],
                            )
                            ho = QC // (2 * N_CORES)
                            nc.gpsimd.dma_start(
                                out=out_p[qc, half * ho:(half + 1) * ho, :],
                                in_=rh[:],
                            )

                    def make_group(trow, ec, rs_after):
                        def _g():
                            pso = psA_p.tile([128, ECH], F32, tag="sc", name="pso")
                            for h in range(HL):
                                nc.tensor.matmul(
                                    pso[:],
                                    attnT[:, h, trow * 128:(trow + 1) * 128],
                                    wo_sb[:, h, ec * ECH:(ec + 1) * ECH],
                                    start=(h == 0),
                                    stop=(h == HL - 1),
                                )
                            ost = ostg_p.tile([128, ECH], BF16, tag="ost", name="ost")
                            nc.vector.tensor_copy(ost[:], pso[:])
                            nc.sync.dma_start(
                                out=partial[qc][(trow % 4) * 128:(trow % 4 + 1) * 128,
                                                ec * ECH:(ec + 1) * ECH],
                                in_=ost[:],
                            )
                            if rs_after is not None:
                                emit_rs(qc, None if rs_after < 0 else rs_after)
                        return _g

                    rows = [qc * 4 + tt for tt in range(QC // 128)]
                    last_slab = qc == N_QC - 1
                    n = len(rows) * N_ECH
                    i = 0
                    for trow in rows:
                        for ec in range(N_ECH):
                            i += 1
                            rs_after = None
                            if i == n // 2:
                                rs_after = 0
                            elif i == n:
                                rs_after = 1
                            oproj_q.append(make_group(trow, ec, rs_after))

                for qc in range(N_QC):
                    qsl = slice(qc * QC, (qc + 1) * QC)
                    si_lo = max(0, 4 * qc - 8)
                    sis = list(range(si_lo, 4 * qc + 4))
                    for hp in range(0, HL, 2):
                        avs, sums = [], []
                        for j in range(2):
                            avs.append(psav_p.tile([128, QC], F32, tag="av", name="av"))
                            sums.append(pssum_p.tile([1, QC], F32, tag="sum", name="sum"))
                        exs = {}

                        # two heads share kT/vnat; their chains interleave so
                        # ACT/DVE latency hides under PE work
                        def emit_scores(si):
                            delta = qc * QC - si * 128
                            masked = delta in MASK_IDX
                            for j in range(2):
                                psc = psA_p.tile([128, QC], F32, tag="sc")
                                nc.tensor.matmul(
                                    psc[:], kT[:, si * 128:(si + 1) * 128],
                                    qT[hp + j][:, qsl], start=True, stop=not masked,
                                )
                                if masked:
                                    nc.tensor.matmul(
                                        psc[:], ident[:], mask_sb[:, MASK_IDX[delta], :],
                                        start=False, stop=True,
                                    )
                                ex = exp_p.tile([128, QC], BF16, tag="ex")
                                nc.scalar.activation(ex[:], psc[:], AF.Exp)
                                exs[(si, j)] = ex

                        def emit_consume(si):
                            first = si == sis[0]
                            last = si == sis[-1]
                            for j in range(2):
                                ex = exs.pop((si, j))
                                nc.tensor.matmul(
                                    sums[j][:], onc[:], ex[:], start=first, stop=last
                                )
                                nc.tensor.matmul(
                                    avs[j][:], vnat[:, si, :], ex[:], start=first, stop=last
                                )

                        emit_scores(sis[0])
                        if len(sis) > 1:
                            emit_scores(sis[1])
                        flush_norm2()
                        for si in sis[2:]:
                            emit_scores(si)
                            emit_consume(si - 2)
                        if len(sis) > 1:
                            emit_consume(sis[-2])
                        emit_consume(sis[-1])

                        def make_norm(hp=hp, avs=avs, sums=sums, qsl=qsl):
                            def _norm():
                                scs, bcps = [], []
                                for j in range(2):
                                    sc_s = stB.tile([1, QC], F32R, tag="rc", name="rc")
                                    nc.scalar.activation(sc_s[:], sums[j][:], AF.Copy)
                                    scs.append(sc_s)
                                for j in range(2):
                                    bcp = psA_p.tile([128, QC], F32, tag="sc", name="bcB")
                                    nc.tensor.matmul(bcp[:], onr[:], scs[j][:], start=True, stop=True)
                                    bcps.append(bcp)
                                rws = []
                                for j in range(2):
                                    rcw = stB.tile([128, QC], F32, tag="rcw", name="rcw")
                                    nc.vector.reciprocal(rcw[:], bcps[j][:])
                                    rws.append(rcw)
                                for j in range(2):
                                    nc.vector.tensor_mul(
                                        attnT[:, hp + j, qsl], avs[j][:], rws[j][:]
                                    )
                            return _norm

                        pending_norm[0] = make_norm()
                    flush_norm()
                    queue_oproj(qc)
                    while oproj_q:
                        emit_oproj_group()

    nc.finalize()
    return nc


_NC_CACHE = None


def _get_nc():
    global _NC_CACHE
    if _NC_CACHE is None:
        _NC_CACHE = _build()
    return _NC_CACHE


def _host_inputs(positions, hidden_states, w_qkv, q_norm_w, k_norm_w, w_o):
    positions = np.asarray(positions)
    hidden_states = np.asarray(hidden_states, dtype=np.float32)
    w_qkv = np.asarray(w_qkv, dtype=np.float32)
    q_norm_w = np.asarray(q_norm_w, dtype=np.float32)
    k_norm_w = np.asarray(k_norm_w, dtype=np.float32)
    w_o = np.asarray(w_o, dtype=np.float32)

    # [ki, tt, ko, j]
    hidT4 = np.ascontiguousarray(
        hidden_states.T.reshape(KO, 128, NT, 128).transpose(1, 2, 0, 3)
    ).astype(ml_dtypes.bfloat16)

    half = D // 2
    inv_freq = 1.0 / (THETA ** (np.arange(half, dtype=np.float32) / half))
    ang = positions.astype(np.float32)[:, None] * inv_freq[None, :]  # [T, 64]
    cos = np.cos(ang).T.astype(np.float32)   # [64, T]
    sin = np.sin(ang).T.astype(np.float32)
    cs2 = np.concatenate([cos, cos], axis=0)          # [128, T]
    sn2s = np.concatenate([-sin, sin], axis=0)        # [128, T]

    mk = np.zeros((len(MASK_DELTAS), 128, QC), np.float32)
    ss = np.arange(128)[:, None]
    ttv = np.arange(QC)[None, :]
    for i, dlt in enumerate(MASK_DELTAS):
        diff = dlt + ttv - ss
        mk[i] = np.where((diff >= 0) & (diff < WINDOW), 0.0, -30000.0).astype(np.float32)
    mk = np.ascontiguousarray(mk.transpose(1, 0, 2)).astype(ml_dtypes.bfloat16)

    qwv = (q_norm_w * (D ** -0.5)).reshape(D, 1).astype(np.float32)
    kwv = k_norm_w.reshape(D, 1).astype(np.float32)
    onc = np.ones((128, 1), ml_dtypes.bfloat16)
    onr = np.ones((1, 128), np.float32)

    in_maps = []
    for c in range(N_CORES):
        wq_c = np.concatenate(
            [
                w_qkv[:, c * HL * D:(c + 1) * HL * D],
                w_qkv[:, H * D + c * D:H * D + (c + 1) * D],
                w_qkv[:, (H + HKV) * D + c * D:(H + HKV) * D + (c + 1) * D],
            ],
            axis=1,
        )
        wq_c = np.ascontiguousarray(wq_c.reshape(KO, 128, (HL + 2) * D).transpose(1, 0, 2)).astype(ml_dtypes.bfloat16)
        wo_c = np.ascontiguousarray(
            w_o[c * HL * D:(c + 1) * HL * D, :].reshape(HL, 128, HID).transpose(1, 0, 2)
        ).astype(ml_dtypes.bfloat16)
        in_maps.append(
            {
                "hidT": hidT4,
                "wq": wq_c,
                "wo": wo_c,
                "cs2": cs2,
                "sn2s": sn2s,
                "masks": mk,
                "qwv": qwv,
                "kwv": kwv,
                "onc": onc,
                "onr": onr,
            }
        )
    return in_maps


def _assemble(results):
    out = np.empty((T, HID), np.float32)
    rows = QC // N_CORES
    half = rows // 2
    for c in range(N_CORES):
        r = np.asarray(results[c]["out"], dtype=np.float32)  # [N_QC, rows, HID]
        for qc in range(N_QC):
            base = qc * QC
            out[base + c * half: base + (c + 1) * half] = r[qc][:half]
            out[base + QC // 2 + c * half: base + QC // 2 + (c + 1) * half] = r[qc][half:]
    return out


def run_spmd(in_maps, trace=False, **kw):
    nc = _get_nc()
    return run_bass_kernel_spmd(nc, in_maps, list(range(N_CORES)), trace=trace, **kw)


def kernel(positions, hidden_states, w_qkv, q_norm_w, k_norm_w, w_o):
    in_maps = _host_inputs(positions, hidden_states, w_qkv, q_norm_w, k_norm_w, w_o)
    last_err = None
    for _ in range(3):
        try:
            res = run_spmd(in_maps)
            return _assemble(res.results)
        except Exception as e:  # rare transient NRT_EXEC_UNIT_UNRECOVERABLE
            last_err = e
    raise last_err


# revision 27
# speedup vs baseline: 1.0403x; 1.0403x over previous
"""Bass/Trainium2 kernel for nn_ExaoneMoEAttention (sliding-window GQA attention).

Strategy (8 NeuronCores, tensor-parallel over heads):
  - core c owns q heads 4c..4c+3 and kv head c (w_qkv column shard [4096, 768]),
    plus w_o rows 512c..512c+512 ([512, 4096]).
  - hidden replicated, host-transposed/blocked; fp32r matmuls throughout.
  - Phase A (QKV proj): per 128-row t-tile, hidT tiles are the stationary
    operand and w_qkv columns the moving operand (qkv lands in [t, c] psum);
    RMSNorm stats via ACT Square+accum_out on the free dim; the normalized
    q/k head tiles are PE-transposed to [d, t] strips resident in SBUF, with
    norm-weight (and softmax scale for q) folded into the transpose
    evacuation; RoPE via host cos/sin tables; v needs no transpose.
  - Phase B: scoresT[s, t] tiles of [128, 512]; sliding window (1024) +
    causal handled block-sparsely (<=12 key tiles per 512-wide q chunk) with
    multiplicative 0/1 masks; softmax without max-subtraction (RMSNorm bounds
    |score| <= sqrt(D)); exp-sum via ones-matmul; two GQA heads pipelined
    together; normalization by broadcast reciprocal, pipelined into the next
    head-pair.
  - o_proj per 512-row slab, then bf16 ReduceScatter(add) over the 8 cores
    per slab (overlaps later slabs); host concatenates the 8 row-shards.
"""

import ml_dtypes
import numpy as np

import concourse.bass as bass
import concourse.mybir as mybir
import concourse.tile as tile
from concourse import bacc
from concourse.bass_utils import run_bass_kernel_spmd
from concourse.masks import make_identity

F32 = mybir.dt.float32
F32R = mybir.dt.float32r
BF16 = mybir.dt.bfloat16
AF = mybir.ActivationFunctionType

N_CORES = 8
T = 2048
HID = 4096
H = 32
HKV = 8
D = 128
EPS = 1e-5
THETA = 1e6
WINDOW = 1024

HL = H // N_CORES          # 4 local q heads
NT = T // 128              # 16 t/s tiles
KO = HID // 128            # 32 k-subtiles in projection
QC = 512                   # q chunk in attention phase
N_QC = T // QC             # 4
ECH = 512                  # o_proj e-chunk
N_ECH = HID // ECH         # 8

MASK_DELTAS = [0, -128, -256, -384, 640, 768, 896, 1024]
MASK_IDX = {d: i for i, d in enumerate(MASK_DELTAS)}


def _build():
    nc = bacc.Bacc(num_devices=N_CORES)

    # hidT4[ki, tt, ko, j] = hidden[tt*128+j, ko*128+ki]
    hidT = nc.declare_dram_parameter("hidT", [128, NT, KO, 128], BF16, isOutput=False)
    wq = nc.declare_dram_parameter("wq", [128, KO, (HL + 2) * D], BF16, isOutput=False)
    wo = nc.declare_dram_parameter("wo", [128, HL, HID], BF16, isOutput=False)
    cs2 = nc.declare_dram_parameter("cs2", [128, T], F32, isOutput=False)
    sn2s = nc.declare_dram_parameter("sn2s", [128, T], F32, isOutput=False)
    masks = nc.declare_dram_parameter("masks", [128, len(MASK_DELTAS), QC], BF16, isOutput=False)
    qwv = nc.declare_dram_parameter("qwv", [D, 1], F32, isOutput=False)
    kwv = nc.declare_dram_parameter("kwv", [D, 1], F32, isOutput=False)
    onc_d = nc.declare_dram_parameter("onc", [128, 1], BF16, isOutput=False)
    onr_d = nc.declare_dram_parameter("onr", [1, 128], F32R, isOutput=False)
    out_p = nc.declare_dram_parameter("out", [N_QC, QC // N_CORES, HID], BF16, isOutput=True)

    with tile.TileContext(nc) as tc:
        with tc.tile_pool(name="persistA", bufs=1) as pA:
            kT = pA.tile([128, T], F32R)                     # rope'd k, [d, s]
            qT = [pA.tile([128, T], F32R, name=f"qT{h}") for h in range(HL)]
            vnat = pA.tile([128, NT, D], BF16)               # v in [s, d] tiles
            onc = pA.tile([128, 1], BF16)
            onr = pA.tile([1, 128], F32R)
            ident = pA.tile([128, 128], BF16)
            make_identity(nc, ident[:])
            nc.sync.dma_start(out=onc[:], in_=onc_d[:])
            nc.sync.dma_start(out=onr[:], in_=onr_d[:])

            # ---------------- Phase A: QKV projection + norm + rope ----------
            with (
                tc.tile_pool(name="wpool", bufs=1) as wpool,
                tc.tile_pool(name="hidp", bufs=3) as hidp,
                tc.tile_pool(name="cspool", bufs=2) as cspool,
                tc.tile_pool(name="tmpA", bufs=6) as tmpA,
                tc.tile_pool(name="stA", bufs=6) as stA,
                tc.tile_pool(name="miscA", bufs=1) as miscA,
                tc.tile_pool(name="psq", bufs=3, space="PSUM") as psq_p,
                tc.tile_pool(name="psvt", bufs=2, space="PSUM") as psvt_p,
            ):
                w_grp = []
                for g in range(4):
                    wt = wpool.tile([128, KO // 4, (HL + 2) * D], BF16, name=f"w{g}")
                    nc.sync.dma_start(out=wt[:], in_=wq[:, g * (KO // 4):(g + 1) * (KO // 4), :])
                    w_grp.append(wt)
                qw_sb = miscA.tile([D, 1], F32)
                kw_sb = miscA.tile([D, 1], F32)
                eps_sb = miscA.tile([128, 1], F32)
                nc.sync.dma_start(out=qw_sb[:], in_=qwv[:])
                nc.sync.dma_start(out=kw_sb[:], in_=kwv[:])
                nc.vector.memset(eps_sb[:], EPS)

                pending_post = []

                def flush_post(keep=0):
                    while len(pending_post) > keep:
                        pending_post.pop(0)()

                for tt in range(NT):
                    tsl = slice(tt * 128, (tt + 1) * 128)
                    hid_t = hidp.tile([128, KO, 128], BF16, tag="hid")
                    nc.sync.dma_start(out=hid_t[:], in_=hidT[:, tt])
                    cs_t = cspool.tile([128, 128], F32, tag="cs")
                    sn_t = cspool.tile([128, 128], F32, tag="sn")
                    nc.sync.dma_start(out=cs_t[:], in_=cs2[:, tsl])
                    nc.sync.dma_start(out=sn_t[:], in_=sn2s[:, tsl])

                    # qkv[t, c] for this t-tile: [128, 512] + [128, 256] psums
                    pq = psq_p.tile([128, 4 * D], F32, tag="pq")
                    pq2 = psq_p.tile([128, 2 * D], F32, tag="pq2")
                    for ko in range(KO):
                        lhsT = hid_t[:, ko, :]
                        wg = w_grp[ko // (KO // 4)]
                        nc.tensor.matmul(
                            pq[:], lhsT, wg[:, ko % (KO // 4), 0:4 * D],
                            start=(ko == 0), stop=(ko == KO - 1),
                        )
                        nc.tensor.matmul(
                            pq2[:], lhsT, wg[:, ko % (KO // 4), 4 * D:6 * D],
                            start=(ko == 0), stop=(ko == KO - 1),
                        )
                    flush_post(keep=1)

                    def make_post(tt=tt, pq=pq, pq2=pq2, tsl=tsl, cs_t=cs_t, sn_t=sn_t):
                        def _post():
                            # stage-parallel across the 5 normed heads so the
                            # ACT/DVE chains pipeline instead of serializing
                            srcs = [pq[:, m * D:(m + 1) * D] for m in range(HL)] + [pq2[:, 0:D]]
                            var, sd, rstd, ev, tp, qd, qsw = [], [], [], [], [], [], []
                            for m in range(HL + 1):
                                sqd = tmpA.tile([128, D], F32, tag="sqd", name="sqd")
                                var.append(stA.tile([128, 1], F32, tag="var", name="var"))
                                nc.scalar.activation(sqd[:], srcs[m], AF.Square, accum_out=var[m][:])
                            for m in range(HL + 1):
                                sd.append(stA.tile([128, 1], F32, tag="sd", name="sd"))
                                nc.scalar.activation(sd[m][:], var[m][:], AF.Sqrt, scale=1.0 / D, bias=eps_sb[:])
                            for m in range(HL + 1):
                                rstd.append(stA.tile([128, 1], F32, tag="rstd", name="rstd"))
                                nc.vector.reciprocal(rstd[m][:], sd[m][:])
                            for m in range(HL + 1):
                                ev.append(tmpA.tile([128, D], BF16, tag="ev", name="ev"))
                                nc.scalar.activation(ev[m][:], srcs[m], AF.Copy, scale=rstd[m][:])
                            for m in range(HL + 1):
                                tp.append(psvt_p.tile([128, 128], BF16, tag="tp", name="tp"))
                                nc.tensor.transpose(tp[m][:], ev[m][:], ident[:])
                            for m in range(HL + 1):
                                qd.append(tmpA.tile([128, D], F32, tag="qd", name="qd"))
                                nc.scalar.activation(
                                    qd[m][:], tp[m][:], AF.Copy,
                                    scale=(qw_sb[:] if m < HL else kw_sb[:]),
                                )
                            for m in range(HL + 1):
                                qsw.append(tmpA.tile([128, D], F32, tag="qsw", name="qsw"))
                                nc.vector.tensor_copy(qsw[m][0:64, :], qd[m][64:128, :])
                                nc.vector.tensor_copy(qsw[m][64:128, :], qd[m][0:64, :])
                            for m in range(HL + 1):
                                nc.vector.tensor_mul(qd[m][:], qd[m][:], cs_t[:])
                                nc.vector.tensor_mul(qsw[m][:], qsw[m][:], sn_t[:])
                                dst = qT[m][:, tsl] if m < HL else kT[:, tsl]
                                nc.vector.tensor_add(dst, qd[m][:], qsw[m][:])
                            nc.vector.tensor_copy(vnat[:, tt, :], pq2[:, D:2 * D])
                        return _post

                    pending_post.append(make_post())
                flush_post()

            # ---------------- Phase B: attention + o_proj + reduce-scatter ---
            with (
                tc.tile_pool(name="persistB", bufs=1) as pB,
                tc.tile_pool(name="exp", bufs=8) as exp_p,
                tc.tile_pool(name="stB", bufs=2) as stB,
                tc.tile_pool(name="ostg", bufs=8) as ostg_p,
                tc.tile_pool(name="cvp", bufs=1) as cvp,
                tc.tile_pool(name="psA", bufs=4, space="PSUM") as psA_p,
                tc.tile_pool(name="psav", bufs=2, space="PSUM") as psav_p,
                tc.tile_pool(name="pssum", bufs=2, space="PSUM") as pssum_p,
                tc.tile_pool(name="dramB", bufs=1, space="DRAM") as dramB,
            ):
                attnT = pB.tile([128, HL, T], BF16)
                wo_sb = pB.tile([128, HL, HID], BF16)
                mask_sb = pB.tile([128, len(MASK_DELTAS), QC], BF16)
                nc.sync.dma_start(out=mask_sb[:], in_=masks[:])
                nc.gpsimd.dma_start(out=wo_sb[:], in_=wo[:])

                partial = [
                    dramB.tile([QC, HID], BF16, name=f"partial{qc}") for qc in range(N_QC)
                ]
                rs_out = [
                    dramB.tile([QC // N_CORES, HID], BF16, name=f"rsout{qc}")
                    for qc in range(N_QC)
                ]
                rs_half = [
                    dramB.tile([QC // (2 * N_CORES), HID], BF16, name=f"rshalf{i}")
                    for i in range(2)
                ]

                pending_norm = [None]

                def flush_norm():
                    if pending_norm[0] is not None:
                        pending_norm[0]()
                        pending_norm[0] = None

                flush_norm2 = flush_norm

                # o_proj emitted as small PE groups, interleaved into the next
                # slab's attention loop as filler work so the PE stream stays
                # dense (keeps the HAM clock-gate warm)
                oproj_q = []

                def emit_oproj_group():
                    if oproj_q:
                        oproj_q.pop(0)()

                def queue_oproj(qc):
                    def emit_rs(qc, half):
                        # half: None = whole slab; 0/1 = 256-row half of the
                        # final slab (lets RS(3a) overlap o_proj(3b), halving
                        # the un-overlapped tail collective)
                        if half is None:
                            nc.gpsimd.collective_compute(
                                "ReduceScatter",
                                mybir.AluOpType.add,
                                replica_groups=[list(range(N_CORES))],
                                ins=[partial[qc][:]],
                                outs=[rs_out[qc][:]],
                            )
                            # same gpsimd queue as the RS, so this wait can't
                            # poison other engines
                            nc.gpsimd.dma_start(out=out_p[qc], in_=rs_out[qc][:])
                        else:
                            hrows = QC // 2
                            nc.gpsimd.collective_compute(
                                "ReduceScatter",
                                mybir.AluOpType.add,
                                replica_groups=[list(range(N_CORES))],
                                ins=[partial[qc][half * hrows:(half + 1) * hrows, :]],
                                outs=[rs_half[half][:]],
                            )
                            ho = QC // (2 * N_CORES)
                            nc.gpsimd.dma_start(
                                out=out_p[qc, half * ho:(half + 1) * ho, :],
                                in_=rs_half[half][:],
                            )

                    def make_group(trow, ec, rs_after):
                        def _g():
                            pso = psA_p.tile([128, ECH], F32, tag="sc", name="pso")
                            for h in range(HL):
                                nc.tensor.matmul(
                                    pso[:],
                                    attnT[:, h, trow * 128:(trow + 1) * 128],
                                    wo_sb[:, h, ec * ECH:(ec + 1) * ECH],
                                    start=(h == 0),
                                    stop=(h == HL - 1),
                                )
                            ost = ostg_p.tile([128, ECH], BF16, tag="ost", name="ost")
                            nc.vector.tensor_copy(ost[:], pso[:])
                            nc.sync.dma_start(
                                out=partial[qc][(trow % 4) * 128:(trow % 4 + 1) * 128,
                                                ec * ECH:(ec + 1) * ECH],
                                in_=ost[:],
                            )
                            if rs_after is not None:
                                emit_rs(qc, None if rs_after < 0 else rs_after)
                        return _g

                    rows = [qc * 4 + tt for tt in range(QC // 128)]
                    last_slab = qc == N_QC - 1
                    n = len(rows) * N_ECH
                    i = 0
                    for trow in rows:
                        for ec in range(N_ECH):
                            i += 1
                            rs_after = None
                            if last_slab and i == n // 2:
                                rs_after = 0
                            elif last_slab and i == n:
                                rs_after = 1
                            elif i == n:
                                rs_after = -1  # whole-slab RS marker
                            oproj_q.append(make_group(trow, ec, rs_after))

                for qc in range(N_QC):
                    qsl = slice(qc * QC, (qc + 1) * QC)
                    si_lo = max(0, 4 * qc - 8)
                    sis = list(range(si_lo, 4 * qc + 4))
                    for hp in range(0, HL, 2):
                        avs, sums = [], []
                        for j in range(2):
                            avs.append(psav_p.tile([128, QC], F32, tag="av", name="av"))
                            sums.append(pssum_p.tile([1, QC], F32, tag="sum", name="sum"))
                        exs = {}

                        # two heads share kT/vnat; their chains interleave so
                        # ACT/DVE latency hides under PE work
                        def emit_scores(si):
                            delta = qc * QC - si * 128
                            masked = delta in MASK_IDX
                            for j in range(2):
                                psc = psA_p.tile([128, QC], F32, tag="sc")
                                nc.tensor.matmul(
                                    psc[:], kT[:, si * 128:(si + 1) * 128],
                                    qT[hp + j][:, qsl], start=True, stop=not masked,
                                )
                                if masked:
                                    nc.tensor.matmul(
                                        psc[:], ident[:], mask_sb[:, MASK_IDX[delta], :],
                                        start=False, stop=True,
                                    )
                                ex = exp_p.tile([128, QC], BF16, tag="ex")
                                nc.scalar.activation(ex[:], psc[:], AF.Exp)
                                exs[(si, j)] = ex

                        def emit_consume(si):
                            first = si == sis[0]
                            last = si == sis[-1]
                            for j in range(2):
                                ex = exs.pop((si, j))
                                nc.tensor.matmul(
                                    sums[j][:], onc[:], ex[:], start=first, stop=last
                                )
                                nc.tensor.matmul(
                                    avs[j][:], vnat[:, si, :], ex[:], start=first, stop=last
                                )

                        emit_scores(sis[0])
                        if len(sis) > 1:
                            emit_scores(sis[1])
                        flush_norm2()
                        for si in sis[2:]:
                            emit_scores(si)
                            emit_consume(si - 2)
                        if len(sis) > 1:
                            emit_consume(sis[-2])
                        emit_consume(sis[-1])

                        def make_norm(hp=hp, avs=avs, sums=sums, qsl=qsl):
                            def _norm():
                                scs, bcps = [], []
                                for j in range(2):
                                    sc_s = stB.tile([1, QC], F32R, tag="rc", name="rc")
                                    nc.scalar.activation(sc_s[:], sums[j][:], AF.Copy)
                                    scs.append(sc_s)
                                for j in range(2):
                                    bcp = psA_p.tile([128, QC], F32, tag="sc", name="bcB")
                                    nc.tensor.matmul(bcp[:], onr[:], scs[j][:], start=True, stop=True)
                                    bcps.append(bcp)
                                rws = []
                                for j in range(2):
                                    rcw = stB.tile([128, QC], F32, tag="rcw", name="rcw")
                                    nc.vector.reciprocal(rcw[:], bcps[j][:])
                                    rws.append(rcw)
                                for j in range(2):
                                    nc.vector.tensor_mul(
                                        attnT[:, hp + j, qsl], avs[j][:], rws[j][:]
                                    )
                            return _norm

                        pending_norm[0] = make_norm()
                    flush_norm()
                    queue_oproj(qc)
                    while oproj_q:
                        emit_oproj_group()

    nc.finalize()
    return nc


_NC_CACHE = None


def _get_nc():
    global _NC_CACHE
    if _NC_CACHE is None:
        _NC_CACHE = _build()
    return _NC_CACHE


def _host_inputs(positions, hidden_states, w_qkv, q_norm_w, k_norm_w, w_o):
    positions = np.asarray(positions)
    hidden_states = np.asarray(hidden_states, dtype=np.float32)
    w_qkv = np.asarray(w_qkv, dtype=np.float32)
    q_norm_w = np.asarray(q_norm_w, dtype=np.float32)
    k_norm_w = np.asarray(k_norm_w, dtype=np.float32)
    w_o = np.asarray(w_o, dtype=np.float32)

    # [ki, tt, ko, j]
    hidT4 = np.ascontiguousarray(
        hidden_states.T.reshape(KO, 128, NT, 128).transpose(1, 2, 0, 3)
    ).astype(ml_dtypes.bfloat16)

    half = D // 2
    inv_freq = 1.0 / (THETA ** (np.arange(half, dtype=np.float32) / half))
    ang = positions.astype(np.float32)[:, None] * inv_freq[None, :]  # [T, 64]
    cos = np.cos(ang).T.astype(np.float32)   # [64, T]
    sin = np.sin(ang).T.astype(np.float32)
    cs2 = np.concatenate([cos, cos], axis=0)          # [128, T]
    sn2s = np.concatenate([-sin, sin], axis=0)        # [128, T]

    mk = np.zeros((len(MASK_DELTAS), 128, QC), np.float32)
    ss = np.arange(128)[:, None]
    ttv = np.arange(QC)[None, :]
    for i, dlt in enumerate(MASK_DELTAS):
        diff = dlt + ttv - ss
        mk[i] = np.where((diff >= 0) & (diff < WINDOW), 0.0, -30000.0).astype(np.float32)
    mk = np.ascontiguousarray(mk.transpose(1, 0, 2)).astype(ml_dtypes.bfloat16)

    qwv = (q_norm_w * (D ** -0.5)).reshape(D, 1).astype(np.float32)
    kwv = k_norm_w.reshape(D, 1).astype(np.float32)
    onc = np.ones((128, 1), ml_dtypes.bfloat16)
    onr = np.ones((1, 128), np.float32)

    in_maps = []
    for c in range(N_CORES):
        wq_c = np.concatenate(
            [
                w_qkv[:, c * HL * D:(c + 1) * HL * D],
                w_qkv[:, H * D + c * D:H * D + (c + 1) * D],
                w_qkv[:, (H + HKV) * D + c * D:(H + HKV) * D + (c + 1) * D],
            ],
            axis=1,
        )
        wq_c = np.ascontiguousarray(wq_c.reshape(KO, 128, (HL + 2) * D).transpose(1, 0, 2)).astype(ml_dtypes.bfloat16)
        wo_c = np.ascontiguousarray(
            w_o[c * HL * D:(c + 1) * HL * D, :].reshape(HL, 128, HID).transpose(1, 0, 2)
        ).astype(ml_dtypes.bfloat16)
        in_maps.append(
            {
                "hidT": hidT4,
                "wq": wq_c,
                "wo": wo_c,
                "cs2": cs2,
                "sn2s": sn2s,
                "masks": mk,
                "qwv": qwv,
                "kwv": kwv,
                "onc": onc,
                "onr": onr,
            }
        )
    return in_maps


def _assemble(results):
    out = np.empty((T, HID), np.float32)
    rows = QC // N_CORES
    half = rows // 2
    for c in range(N_CORES):
        r = np.asarray(results[c]["out"], dtype=np.float32)  # [N_QC, rows, HID]
        for qc in range(N_QC - 1):
            out[qc * QC + c * rows: qc * QC + (c + 1) * rows] = r[qc]
        # final slab was reduce-scattered as two 256-row halves
        qc = N_QC - 1
        base = qc * QC
        out[base + c * half: base + (c + 1) * half] = r[qc][:half]
        out[base + QC // 2 + c * half: base + QC // 2 + (c + 1) * half] = r[qc][half:]
    return out


def run_spmd(in_maps, trace=False, **kw):
    nc = _get_nc()
    return run_bass_kernel_spmd(nc, in_maps, list(range(N_CORES)), trace=trace, **kw)


def kernel(positions, hidden_states, w_qkv, q_norm_w, k_norm_w, w_o):
    in_maps = _host_inputs(positions, hidden_states, w_qkv, q_norm_w, k_norm_w, w_o)
    last_err = None
    for _ in range(3):
        try:
            res = run_spmd(in_maps)
            return _assemble(res.results)
        except Exception as e:  # rare transient NRT_EXEC_UNIT_UNRECOVERABLE
            last_err = e
    raise last_err
